# revision 58
# baseline (speedup 1.0000x reference)
"""Multi-head attention + layernorm Bass kernel for Trainium2, 8 cores.

Problem: B=8, S=1024, D=768, H=12 heads x DH=64, key-padding mask, softmax,
output projection, layernorm.  Sharding: pure data parallelism -- one batch
element per NeuronCore, no collectives.

v2 schedule (265us -> 204us): ACT (exp, (N+352)/1.2 ns) and the PE are
co-critical at ~110-130us each; everything is arranged so neither idles:
  - key-padding mask folded into V on the host (v_row *= em_k, em in {0,1},
    incl. the ones-denominator column), so exp needs no bias and no mask
    tensor ever reaches the scores path.
  - per-pair attention slots emit [scores][exp][one lookahead unit][ctx
    two slots behind]; lookahead units are the next pair's q/k projection
    halves and the upcoming quad's v blocks, each an atomic
    alloc->MMs->consumer group on the 2x1-bank 'proj' psum tag, so the PE
    FIFO always has ready work while exp runs and pair boundaries never
    drain the pipeline.
  - softmax denominators ride row 64 of the ctx accumulators; at each iblk
    boundary the release chain (denominator rows + ct casts) runs at high
    priority, then reciprocal_approx_fast + partition_broadcast (gpsimd
    DGE) + a deferred DVE normalize replace the old fp32 selector matmuls
    and the 3.4us DVE reciprocal that used to stall the PE.
  - v-projection bias via a K=1 rank-one matmul so the single DVE pass per
    v-tile also applies the mask scaling (psv * em).
  - out projection: pso evicted to SBUF with one DVE copy so the 2-deep
    'st' rotation never stalls; LN stats on DVE, z-pass on ACT, beta-add
    on GPSIMD; output DMA alternates sync/scalar queues.
  - head: wq0/wk0 then x^T (c-chunk, query-half) pieces round-robin on the
    three DMA queues (sync/scalar/gpsimd), first exp by ~25us; gamma/beta
    loaded [1,D] and partition-broadcast on chip.
Known hardware quirks honored: gpsimd software-DGE must not carry bulk
weight DMAs (serializes with Q7 compute + lib swaps); partition_broadcast
and gpsimd tensor ops only address base partition 0; custom-DVE ops cannot
read PSUM at base partition 64 (hence the drow staging copies).
"""

import numpy as np

B, S, D, H, DH = 8, 1024, 768, 12, 64
NPAIR, NQUAD = H // 2, H // 4
SBLK = S // 128      # 8 key/row chunks
DCH = D // 128       # 6 contraction chunks
LN_EPS = 1e-5

_PROGRAM = None


def _build_program():
    import concourse.bass as bass
    from concourse import bacc
    import concourse.tile as tile
    import concourse.mybir as mybir
    from contextlib import ExitStack

    F32 = mybir.dt.float32
    F16 = mybir.dt.float16
    AF = mybir.ActivationFunctionType

    nc = bacc.Bacc("TRN2", target_bir_lowering=False)

    xt_d = nc.dram_tensor("xt", [128, DCH * S], F16, kind="ExternalInput")
    wq_d = nc.dram_tensor("wq", [NPAIR, 128, DCH * 128], F16, kind="ExternalInput")
    wk_d = nc.dram_tensor("wk", [NPAIR, 128, DCH * 128], F16, kind="ExternalInput")
    wv_d = nc.dram_tensor("wv", [NQUAD, 128, DCH * 260], F16, kind="ExternalInput")
    wo_d = nc.dram_tensor("wo", [128, DCH * D], F16, kind="ExternalInput")
    bqk_d = nc.dram_tensor("bqk", [128, 2 * NPAIR], F32, kind="ExternalInput")
    bvr_d = nc.dram_tensor("bvr", [1, NQUAD * 260], F16, kind="ExternalInput")
    em_d = nc.dram_tensor("em", [128, SBLK], F32, kind="ExternalInput")
    gamma_d = nc.dram_tensor("gamma", [1, D], F32, kind="ExternalInput")
    beta_d = nc.dram_tensor("beta", [1, D], F32, kind="ExternalInput")
    onesr_d = nc.dram_tensor("onesr", [1, 128], F16, kind="ExternalInput")
    bor_d = nc.dram_tensor("bor", [1, D], F16, kind="ExternalInput")
    out_d = nc.dram_tensor("out", [S, D], F32, kind="ExternalOutput")

    with tile.TileContext(nc) as tc, ExitStack() as ctx:
        const = ctx.enter_context(tc.tile_pool(name="const", bufs=1))
        xt_p = ctx.enter_context(tc.tile_pool(name="xt_p", bufs=1))
        w_p = ctx.enter_context(tc.tile_pool(name="w_p", bufs=1))
        qk_p = ctx.enter_context(tc.tile_pool(name="qk_p", bufs=1))
        v_p = ctx.enter_context(tc.tile_pool(name="v_p", bufs=1))
        e_p = ctx.enter_context(tc.tile_pool(name="e_p", bufs=1))
        cx_p = ctx.enter_context(tc.tile_pool(name="cx_p", bufs=1))
        z_p = ctx.enter_context(tc.tile_pool(name="z_p", bufs=1))
        # 8 PSUM banks: proj 1x[128,1024] (2) + st 2x[128,1024] (4) +
        # cx 2x[65,512] (2)
        ps = ctx.enter_context(tc.tile_pool(name="ps", bufs=1, space="PSUM"))

        # ---- x^T: (c-chunk, query-half) piece DMAs over all three DMA
        # queues; half-0 pieces land first so the first projections and
        # scores start ~8us earlier ----
        xt_t = xt_p.tile([128, DCH, S], F16, name="xt_t")
        xt = [xt_t[:, c, :] for c in range(DCH)]

        # dram is h-major [128, 2, DCH, 512]: a c-pair per half is one
        # contiguous 2KB-line read (strided write into the c-major tile)
        def load_xt(cp, eng, h):
            eng.dma_start(
                out=xt_t[:, 2 * cp:2 * cp + 2, h * 512:(h + 1) * 512],
                in_=xt_d[:, h * 3 * S + cp * 1024:
                         h * 3 * S + (cp + 1) * 1024])

        # ---- weights: scalar queue gets pair-0 + v first ----
        wq_ts, wk_ts, wv_ts = [None] * NPAIR, [None] * NPAIR, [None] * NQUAD

        def load_wq(p, eng):
            wqp = w_p.tile([128, DCH, 128], F16, name=f"wq{p}", bufs=1)
            eng.dma_start(out=wqp, in_=wq_d[p])
            wq_ts[p] = [wqp[:, c, :] for c in range(DCH)]

        def load_wk(p, eng):
            wkp = w_p.tile([128, DCH, 128], F16, name=f"wk{p}", bufs=1)
            eng.dma_start(out=wkp, in_=wk_d[p])
            wk_ts[p] = [wkp[:, c, :] for c in range(DCH)]

        def load_wqk(p, eng):
            load_wq(p, eng)
            load_wk(p, eng)

        def load_wv(q, eng):
            wvq = w_p.tile([128, DCH, 260], F16, name=f"wv{q}", bufs=1)
            eng.dma_start(out=wvq, in_=wv_d[q])
            wv_ts[q] = [wvq[:, c, :] for c in range(DCH)]

        # ---- small constants first on gpsimd queue (it is free early;
        # all later gpsimd work is broadcasts only) ----
        bqk_t = const.tile([128, 2 * NPAIR], F32)
        nc.gpsimd.dma_start(out=bqk_t, in_=bqk_d[:, :])
        em_t = const.tile([128, SBLK], F32)
        nc.gpsimd.dma_start(out=em_t, in_=em_d[:, :])
        onesr_t = const.tile([1, 128], F16)
        nc.gpsimd.dma_start(out=onesr_t, in_=onesr_d[:, :])
        bvr_t = const.tile([1, NQUAD * 260], F16)
        nc.gpsimd.dma_start(out=bvr_t, in_=bvr_d[:, :])
        bor_t = const.tile([1, D], F16)
        nc.gpsimd.dma_start(out=bor_t, in_=bor_d[:, :])
        gamma1 = const.tile([1, D], F32)
        nc.gpsimd.dma_start(out=gamma1, in_=gamma_d[:, :])
        beta1 = const.tile([1, D], F32)
        nc.gpsimd.dma_start(out=beta1, in_=beta_d[:, :])
        eps_t = const.tile([128, 1], F32)
        nc.vector.memset(eps_t, LN_EPS)
        warm_t = const.tile([128, 512], F16)
        nc.vector.memset(warm_t, 0.0)

        # critical-path first: wq0/wk0 + half-0 x pieces round-robin, then
        # half-1 pieces, then the rest by need
        load_wq(0, nc.sync)
        load_wk(0, nc.scalar)
        load_xt(0, nc.sync, 0)
        load_xt(1, nc.scalar, 0)
        load_xt(2, nc.gpsimd, 0)
        load_xt(0, nc.sync, 1)
        load_xt(1, nc.scalar, 1)
        load_xt(2, nc.gpsimd, 1)
        load_wv(0, nc.scalar)
        load_wqk(1, nc.gpsimd)
        load_wv(1, nc.scalar)
        load_wqk(2, nc.sync)
        load_wqk(3, nc.gpsimd)
        load_wv(2, nc.scalar)
        load_wqk(4, nc.sync)
        load_wqk(5, nc.scalar)
        woa = w_p.tile([128, DCH, D], F16, name="woa", bufs=1)
        nc.sync.dma_start(out=woa[:, 0:3, :], in_=wo_d[:, :3 * D])
        nc.scalar.dma_start(out=woa[:, 3:6, :], in_=wo_d[:, 3 * D:])
        wo_t = [woa[:, c, :] for c in range(DCH)]

        gamma_t = const.tile([128, D], F32)
        nc.gpsimd.partition_broadcast(gamma_t, gamma1)
        beta_t = const.tile([128, D], F32)
        nc.gpsimd.partition_broadcast(beta_t, beta1)

        # ---------- emission helpers ----------
        v_sb = {}   # (quad, sblk) -> [128, 260] f16  (em-scaled, bias incl.)

        def emit_v_block(q, s):
            """V projection for one (quad, s-block): 6 K=128 MMs + rank-one
            bias + one DVE pass (psv * em -> f16)."""
            psv = ps.tile([128, 260], F32, name="psv", tag="proj", bufs=2,
                          padded_shape=[128, 512])
            wv_t = wv_ts[q]
            for c in range(DCH):
                nc.tensor.matmul(psv, xt[c][:, s * 128:(s + 1) * 128], wv_t[c],
                                 start=(c == 0), stop=False)
            nc.tensor.matmul(psv, onesr_t,
                             bvr_t[:, q * 260:(q + 1) * 260],
                             start=False, stop=True)
            vt = v_p.tile([128, 260], F16, name="v_sb", bufs=3 * SBLK)
            nc.vector.tensor_scalar_mul(out=vt, in0=psv,
                                        scalar1=em_t[:, s:s + 1])
            v_sb[(q, s)] = vt

        qt_sb, kt_sb = [None] * NPAIR, [None] * NPAIR

        def emit_proj_half(p, which, half):
            """One half (512 cols) of psq/psk: 6 MMs into an own 1-bank
            psum tile + DVE bias-add into the qt/kt f16 half."""
            w_t = wq_ts[p] if which == "q" else wk_ts[p]
            pst = ps.tile([128, 512], F32,
                          name=("psq" if which == "q" else "psk"),
                          tag="proj", bufs=2)
            for c in range(DCH):
                nc.tensor.matmul(
                    pst, w_t[c], xt[c][:, half * 512:(half + 1) * 512],
                    start=(c == 0), stop=(c == DCH - 1))
            if half == 0:
                t = qk_p.tile([128, S], F16,
                              name=("qt_sb" if which == "q" else "kt_sb"),
                              bufs=4)
                if which == "q":
                    qt_sb[p] = t
                else:
                    kt_sb[p] = t
            t = qt_sb[p] if which == "q" else kt_sb[p]
            col = p if which == "q" else NPAIR + p
            nc.vector.tensor_scalar_add(
                out=t[:, half * 512:(half + 1) * 512], in0=pst,
                scalar1=bqk_t[:, col:col + 1])

        def lookahead_units(p):
            """PE work to interleave into pair p's slot stream, one unit per
            slot.  V tiles for quad q land during pairs 2q-1 / 2q-2; pair 0
            finishes quad 0 (s>=1) with a one-slot lead on its own ctx."""
            units = []
            if p == 0:
                units.append(lambda: emit_proj_half(0, "q", 1))
                units.append(lambda: emit_proj_half(0, "k", 1))
                for s in range(4, SBLK):
                    units.append(lambda s=s: emit_v_block(0, s))
            if 0 < p < 5:
                q = (p + 1) // 2
                s0 = 4 * ((p + 1) % 2)
                for s in range(s0, s0 + 4):
                    units.append(lambda q=q, s=s: emit_v_block(q, s))
            if p + 1 < NPAIR:
                for which in ("q", "k"):
                    for half in range(2):
                        units.append(
                            lambda w=which, h=half: emit_proj_half(
                                p + 1, w, h))
            return units

        def emit_proj_h0_pair0():
            # q/k half-0 for pair 0 with c-chunks interleaved, so both
            # projections stream concurrently as x pieces arrive
            psts = {}
            for which in ("q", "k"):
                psts[which] = ps.tile([128, 512], F32, name="ps" + which,
                                      tag="proj", bufs=2)
            for c in range(DCH):
                for which in ("q", "k"):
                    w_t = wq_ts[0] if which == "q" else wk_ts[0]
                    nc.tensor.matmul(
                        psts[which], w_t[c], xt[c][:, 0:512],
                        start=(c == 0), stop=(c == DCH - 1))
            for which in ("q", "k"):
                t = qk_p.tile([128, S], F16,
                              name=("qt_sb" if which == "q" else "kt_sb"),
                              bufs=4)
                if which == "q":
                    qt_sb[0] = t
                else:
                    kt_sb[0] = t
                col = 0 if which == "q" else NPAIR
                nc.vector.tensor_scalar_add(
                    out=t[:, 0:512], in0=psts[which],
                    scalar1=bqk_t[:, col:col + 1])

        # ---- HAM warm-up: dummy matmuls on memset data keep the PE clock
        # at 8/8 through the initial DMA wait, so the first real
        # projections run at full speed (results are never read) ----
        warm_ps = ps.tile([128, 512], F32, name="warm_ps", tag="proj",
                          bufs=2)
        for _ in range(30):
            nc.tensor.matmul(warm_ps, warm_t[:, 0:128], warm_t,
                             start=True, stop=True)

        # ---- prologue: pair 0 projections (half-0 first) + the v tiles
        # that only need xt half-0 (they fill the DMA wait); q/k half-1
        # moves into pair 0's unit stream ----
        emit_proj_h0_pair0()
        for s in range(4):
            emit_v_block(0, s)

        # ---- pair loop ----
        ctxt = []           # per pair [128, 1024] f16 ctx^T (unnormalized
                            # at first; normalized in place by gpsimd)
        pending = []        # deferred (fn) emissions, flushed at slot starts

        for p in range(NPAIR):
            qt, kt = qt_sb[p], kt_sb[p]
            quad, l0 = divmod(2 * p, 4)

            ct = cx_p.tile([128, S], F16, name="ctxt", bufs=NPAIR)
            ctxt.append(ct)

            units = lookahead_units(p)
            ui = 0
            for iblk in range(2):
                pcx = [ps.tile([65, 512], F32, name="pscx", tag="cx", bufs=2)
                       for _ in range(2)]
                ets = [None] * SBLK

                def emit_ctx(j, pcx=pcx, ets=ets, quad=quad, l0=l0):
                    for idx in range(2):
                        vsl = v_sb[(quad, j)][:, (l0 + idx) * 65:
                                              (l0 + idx + 1) * 65]
                        nc.tensor.matmul(pcx[idx], vsl,
                                         ets[j][:, idx * 512:(idx + 1) * 512],
                                         start=(j == 0), stop=(j == SBLK - 1))

                for j in range(SBLK):
                    # flush deferred (previous iblk's normalize) work; at
                    # j==2 the broadcast DMA has had time to land
                    if j == 2:
                        while pending:
                            pending.pop(0)()
                    # scores for (iblk, j): two heads row-tiled
                    pst = ps.tile([128, 1024], F32, name="psst", tag="st",
                                  bufs=2)
                    nc.tensor.matmul(
                        pst[:, 0:512], kt[0:64, j * 128:(j + 1) * 128],
                        qt[0:64, iblk * 512:(iblk + 1) * 512],
                        start=True, stop=True, tile_position=(0, 0))
                    nc.tensor.matmul(
                        pst[:, 512:1024], kt[64:128, j * 128:(j + 1) * 128],
                        qt[64:128, iblk * 512:(iblk + 1) * 512],
                        start=True, stop=True, tile_position=(64, 0))
                    # exp straight from PSUM, no bias (mask lives in V)
                    et = e_p.tile([128, 1024], F16, name="expt", bufs=6)
                    nc.scalar.activation(et, pst, AF.Exp)
                    ets[j] = et
                    # lookahead PE work rides the exp latency (slot 7's unit
                    # is deferred past the epilogue)
                    if ui < len(units) and j < SBLK - 1:
                        units[ui]()
                        ui += 1
                    # ctx runs two slots behind so boundary DVE copies never
                    # block the scores/exp stream
                    if j > 1:
                        emit_ctx(j - 2)
                emit_ctx(SBLK - 2)
                emit_ctx(SBLK - 1)
                # iblk epilogue: denominators + ctx copy-out (release PSUM
                # fast); normalization is deferred off the critical path.
                # pcx release chain first (drow rows + ct casts), with a
                # priority boost over same-window unit DVE work; the recip
                # only feeds the deferred normalize, so it runs after
                drow = z_p.tile([1, 1024], F32, name="drow", bufs=4)
                with tc.high_priority(offset=40):
                    for idx in range(2):
                        nc.vector.tensor_copy(
                            out=drow[0:1, idx * 512:(idx + 1) * 512],
                            in_=pcx[idx][64:65, :])
                    for idx in range(2):
                        nc.vector.tensor_copy(
                            out=ct[idx * 64:(idx + 1) * 64,
                                   iblk * 512:(iblk + 1) * 512],
                            in_=pcx[idx][0:64, :])
                rinv = z_p.tile([1, 1024], F32, name="rinv", bufs=4)
                nc.vector.reciprocal_approx_fast(out=rinv, in_=drow)
                if p == NPAIR - 1:
                    # pair 5 has no lookahead units: a few dummy matmuls
                    # cover the boundary DVE copies so the PE (and HAM)
                    # never go idle while the release chain runs
                    dps = ps.tile([128, 512], F32, name="dummy_ps",
                                  tag="proj", bufs=2)
                    for _ in range(4):
                        nc.tensor.matmul(dps, warm_t[:, 0:128], warm_t,
                                         start=True, stop=True)

                def norm_chain(ct=ct, rinv=rinv, iblk=iblk):
                    bc = z_p.tile([128, 1024], F32, name="bc", bufs=3)
                    nc.gpsimd.partition_broadcast(bc, rinv)
                    for idx in range(2):
                        csl = ct[idx * 64:(idx + 1) * 64,
                                 iblk * 512:(iblk + 1) * 512]
                        nc.vector.tensor_mul(
                            out=csl, in0=csl,
                            in1=bc[idx * 64:(idx + 1) * 64,
                                   idx * 512:(idx + 1) * 512])
                pending.append(norm_chain)

            # drain remaining lookahead units at pair end
            while ui < len(units):
                units[ui]()
                ui += 1

        while pending:
            pending.pop(0)()

        # ---- output projection + layernorm, per row block ----
        for s in range(SBLK):
            # st tag double-buffers pso; the LN chain (~3us) fits within
            # two outproj slots so depth 2 does not stall
            pso = ps.tile([128, D], F32, name="pso", tag="st", bufs=2,
                          padded_shape=[128, 1024])
            for d0, d1 in ((0, 512), (512, 768)):
                for p in range(NPAIR):
                    nc.tensor.matmul(
                        pso[:, d0:d1],
                        ctxt[p][:, s * 128:(s + 1) * 128],
                        wo_t[p][:, d0:d1],
                        start=(p == 0), stop=False)
                # + bo via a K=1 rank-one update: ones_col x bo_row
                nc.tensor.matmul(pso[:, d0:d1], onesr_t, bor_t[:, d0:d1],
                                 start=False, stop=True)
            # evict psum fast (one DVE copy) so pso depth-2 never stalls,
            # then run the whole LN chain off the SBUF copy
            z0 = z_p.tile([128, D], F32, name="z0_sb", bufs=3)
            nc.vector.tensor_copy(out=z0, in_=pso)
            stats = z_p.tile([128, 3, 6], F32, name="stats", bufs=3)
            for g in range(3):
                nc.vector.bn_stats(out=stats[:, g, :],
                                   in_=z0[:, g * 256:(g + 1) * 256])
            mv = z_p.tile([128, 2], F32, name="mv", bufs=3)
            nc.vector.bn_aggr(out=mv, in_=stats)
            stdv = z_p.tile([128, 1], F32, name="stdv", bufs=3)
            nc.scalar.activation(stdv, mv[:, 1:2], AF.Sqrt, bias=eps_t)
            rstd = z_p.tile([128, 1], F32, name="rstd", bufs=3)
            nc.vector.reciprocal(out=rstd, in_=stdv)
            nmr = z_p.tile([128, 1], F32, name="nmr", bufs=3)
            nc.vector.tensor_scalar(out=nmr, in0=mv[:, 0:1], scalar1=rstd,
                                    scalar2=-1.0, op0=mybir.AluOpType.mult,
                                    op1=mybir.AluOpType.mult)
            z = z_p.tile([128, D], F32, name="z_sb", bufs=3)
            nc.scalar.activation(z, z0, AF.Identity, bias=nmr, scale=rstd)
            nc.vector.tensor_mul(out=z, in0=z, in1=gamma_t)
            nc.gpsimd.tensor_add(out=z, in0=z, in1=beta_t)
            eng = (nc.sync, nc.scalar)[s % 2]
            eng.dma_start(out=out_d[s * 128:(s + 1) * 128, :], in_=z)

    nc.compile()
    return nc


def _host_inputs(inputs):
    x = np.asarray(inputs["input_tensor"], np.float32)
    mask = np.asarray(inputs["attention_mask"])
    Wq = np.asarray(inputs["Wq"], np.float32)
    bq = np.asarray(inputs["bq"], np.float32)
    Wk = np.asarray(inputs["Wk"], np.float32)
    bk = np.asarray(inputs["bk"], np.float32)
    Wv = np.asarray(inputs["Wv"], np.float32)
    bv = np.asarray(inputs["bv"], np.float32)
    Wo = np.asarray(inputs["Wo"], np.float32)
    bo = np.asarray(inputs["bo"], np.float32)
    gamma = np.asarray(inputs["gamma"], np.float32)
    beta = np.asarray(inputs["beta"], np.float32)

    scale = np.float32(1.0 / np.sqrt(DH))
    wq_flat = np.ascontiguousarray(
        (Wq * scale).transpose(1, 0, 2).reshape(D, D))
    wk_flat = np.ascontiguousarray(Wk.transpose(1, 0, 2).reshape(D, D))
    bq_s = (bq * scale).reshape(D)
    bk_s = bk.reshape(D)

    wv_aug = np.zeros((D, NQUAD * 260), np.float32)
    bv_aug = np.zeros((1, NQUAD * 260), np.float32)
    for h in range(H):
        q, l = divmod(h, 4)
        base = q * 260 + l * 65
        wv_aug[:, base:base + 64] = Wv[h]
        bv_aug[0, base:base + 64] = bv[h]
        bv_aug[0, base + 64] = 1.0

    bqk = np.zeros((128, 2 * NPAIR), np.float32)
    for p in range(NPAIR):
        bqk[:, p] = bq_s[p * 128:(p + 1) * 128]
        bqk[:, NPAIR + p] = bk_s[p * 128:(p + 1) * 128]

    def sbuf_layout(w, width):
        # [D, n*width] -> [n, 128, DCH*width]: partition-major per tile
        n = w.shape[1] // width
        return np.ascontiguousarray(
            w.reshape(DCH, 128, n, width).transpose(2, 1, 0, 3).reshape(
                n, 128, DCH * width).astype(np.float16))

    shared = {
        "wq": sbuf_layout(wq_flat, 128), "wk": sbuf_layout(wk_flat, 128),
        "wv": sbuf_layout(wv_aug, 260),
        "wo": sbuf_layout(np.ascontiguousarray(Wo), D)[0],
        "bqk": bqk, "bvr": bv_aug.astype(np.float16),
        "gamma": gamma.reshape(1, D), "beta": beta.reshape(1, D),
        "onesr": np.ones((1, 128), np.float16),
        "bor": bo.reshape(1, D).astype(np.float16),
    }
    in_maps = []
    for b in range(B):
        em = mask[b].astype(np.float32)  # 1.0 keep, 0.0 masked
        in_maps.append({
            **shared,
            "xt": np.ascontiguousarray(
                x[b].T.reshape(DCH, 128, S).transpose(1, 0, 2).reshape(
                    128, DCH, 2, 512).transpose(0, 2, 1, 3).reshape(
                    128, DCH * S).astype(np.float16)),
            "em": np.ascontiguousarray(em.reshape(SBLK, 128).T),
        })
    return in_maps


def _get_program():
    global _PROGRAM
    if _PROGRAM is None:
        _PROGRAM = _build_program()
    return _PROGRAM


def kernel(**inputs):
    from concourse.bass_utils import run_bass_kernel_spmd

    nc = _get_program()
    in_maps = _host_inputs(inputs)
    res = run_bass_kernel_spmd(nc, in_maps, list(range(B)))
    return np.stack([res.results[b]["out"] for b in range(B)], axis=0)


if __name__ == "__main__":
    rng = np.random.default_rng(0)
    demo = {
        "input_tensor": rng.standard_normal((B, S, D)).astype(np.float32),
        "attention_mask": np.ones((B, S), bool),
        "Wq": rng.standard_normal((H, D, DH)).astype(np.float32) * 0.03,
        "bq": rng.standard_normal((H, DH)).astype(np.float32) * 0.03,
        "Wk": rng.standard_normal((H, D, DH)).astype(np.float32) * 0.03,
        "bk": rng.standard_normal((H, DH)).astype(np.float32) * 0.03,
        "Wv": rng.standard_normal((H, D, DH)).astype(np.float32) * 0.03,
        "bv": rng.standard_normal((H, DH)).astype(np.float32) * 0.03,
        "Wo": rng.standard_normal((D, D)).astype(np.float32) * 0.03,
        "bo": rng.standard_normal((D,)).astype(np.float32) * 0.03,
        "gamma": np.ones((D,), np.float32),
        "beta": np.zeros((D,), np.float32),
    }
    out = kernel(**demo)
    print("kernel ran, out shape", out.shape, "finite:", np.isfinite(out).all())


# revision 60
# speedup vs baseline: 1.0134x; 1.0134x over previous
"""Multi-head attention + layernorm Bass kernel for Trainium2, 8 cores.

Problem: B=8, S=1024, D=768, H=12 heads x DH=64, key-padding mask, softmax,
output projection, layernorm.  Sharding: pure data parallelism -- one batch
element per NeuronCore, no collectives.

v2 schedule (265us -> 204us): ACT (exp, (N+352)/1.2 ns) and the PE are
co-critical at ~110-130us each; everything is arranged so neither idles:
  - key-padding mask folded into V on the host (v_row *= em_k, em in {0,1},
    incl. the ones-denominator column), so exp needs no bias and no mask
    tensor ever reaches the scores path.
  - per-pair attention slots emit [scores][exp][one lookahead unit][ctx
    two slots behind]; lookahead units are the next pair's q/k projection
    halves and the upcoming quad's v blocks, each an atomic
    alloc->MMs->consumer group on the 2x1-bank 'proj' psum tag, so the PE
    FIFO always has ready work while exp runs and pair boundaries never
    drain the pipeline.
  - softmax denominators ride row 64 of the ctx accumulators; at each iblk
    boundary the release chain (denominator rows + ct casts) runs at high
    priority, then reciprocal_approx_fast + partition_broadcast (gpsimd
    DGE) + a deferred DVE normalize replace the old fp32 selector matmuls
    and the 3.4us DVE reciprocal that used to stall the PE.
  - v-projection bias via a K=1 rank-one matmul so the single DVE pass per
    v-tile also applies the mask scaling (psv * em).
  - out projection: pso evicted to SBUF with one DVE copy so the 2-deep
    'st' rotation never stalls; LN stats on DVE, z-pass on ACT, beta-add
    on GPSIMD; output DMA alternates sync/scalar queues.
  - head: wq0/wk0 then x^T (c-chunk, query-half) pieces round-robin on the
    three DMA queues (sync/scalar/gpsimd), first exp by ~25us; gamma/beta
    loaded [1,D] and partition-broadcast on chip.
Known hardware quirks honored: gpsimd software-DGE must not carry bulk
weight DMAs (serializes with Q7 compute + lib swaps); partition_broadcast
and gpsimd tensor ops only address base partition 0; custom-DVE ops cannot
read PSUM at base partition 64 (hence the drow staging copies).
"""

import numpy as np

B, S, D, H, DH = 8, 1024, 768, 12, 64
NPAIR, NQUAD = H // 2, H // 4
SBLK = S // 128      # 8 key/row chunks
DCH = D // 128       # 6 contraction chunks
LN_EPS = 1e-5

_PROGRAM = None


def _build_program():
    import concourse.bass as bass
    from concourse import bacc
    import concourse.tile as tile
    import concourse.mybir as mybir
    from contextlib import ExitStack

    F32 = mybir.dt.float32
    F16 = mybir.dt.float16
    AF = mybir.ActivationFunctionType

    nc = bacc.Bacc("TRN2", target_bir_lowering=False)

    xt_d = nc.dram_tensor("xt", [128, DCH * S], F16, kind="ExternalInput")
    wq_d = nc.dram_tensor("wq", [NPAIR, 128, DCH * 128], F16, kind="ExternalInput")
    wk_d = nc.dram_tensor("wk", [NPAIR, 128, DCH * 128], F16, kind="ExternalInput")
    wv_d = nc.dram_tensor("wv", [NQUAD, 128, DCH * 260], F16, kind="ExternalInput")
    wo_d = nc.dram_tensor("wo", [128, DCH * D], F16, kind="ExternalInput")
    bqk_d = nc.dram_tensor("bqk", [128, 2 * NPAIR], F32, kind="ExternalInput")
    bvr_d = nc.dram_tensor("bvr", [1, NQUAD * 260], F16, kind="ExternalInput")
    em_d = nc.dram_tensor("em", [128, SBLK], F32, kind="ExternalInput")
    gamma_d = nc.dram_tensor("gamma", [1, D], F32, kind="ExternalInput")
    beta_d = nc.dram_tensor("beta", [1, D], F32, kind="ExternalInput")
    onesr_d = nc.dram_tensor("onesr", [1, 128], F16, kind="ExternalInput")
    bor_d = nc.dram_tensor("bor", [1, D], F16, kind="ExternalInput")
    out_d = nc.dram_tensor("out", [S, D], F32, kind="ExternalOutput")

    with tile.TileContext(nc) as tc, ExitStack() as ctx:
        const = ctx.enter_context(tc.tile_pool(name="const", bufs=1))
        xt_p = ctx.enter_context(tc.tile_pool(name="xt_p", bufs=1))
        w_p = ctx.enter_context(tc.tile_pool(name="w_p", bufs=1))
        qk_p = ctx.enter_context(tc.tile_pool(name="qk_p", bufs=1))
        v_p = ctx.enter_context(tc.tile_pool(name="v_p", bufs=1))
        e_p = ctx.enter_context(tc.tile_pool(name="e_p", bufs=1))
        cx_p = ctx.enter_context(tc.tile_pool(name="cx_p", bufs=1))
        z_p = ctx.enter_context(tc.tile_pool(name="z_p", bufs=1))
        # 8 PSUM banks: proj 1x[128,1024] (2) + st 2x[128,1024] (4) +
        # cx 2x[65,512] (2)
        ps = ctx.enter_context(tc.tile_pool(name="ps", bufs=1, space="PSUM"))

        # ---- x^T: (c-chunk, query-half) piece DMAs over all three DMA
        # queues; half-0 pieces land first so the first projections and
        # scores start ~8us earlier ----
        xt_t = xt_p.tile([128, DCH, S], F16, name="xt_t")
        xt = [xt_t[:, c, :] for c in range(DCH)]

        def load_xt(cs, eng, h):
            for c in cs:
                eng.dma_start(out=xt_t[:, c, h * 512:(h + 1) * 512],
                              in_=xt_d[:, c * S + h * 512:
                                       c * S + (h + 1) * 512])

        # ---- weights: scalar queue gets pair-0 + v first ----
        wq_ts, wk_ts, wv_ts = [None] * NPAIR, [None] * NPAIR, [None] * NQUAD

        def load_wq(p, eng):
            wqp = w_p.tile([128, DCH, 128], F16, name=f"wq{p}", bufs=1)
            eng.dma_start(out=wqp, in_=wq_d[p])
            wq_ts[p] = [wqp[:, c, :] for c in range(DCH)]

        def load_wk(p, eng):
            wkp = w_p.tile([128, DCH, 128], F16, name=f"wk{p}", bufs=1)
            eng.dma_start(out=wkp, in_=wk_d[p])
            wk_ts[p] = [wkp[:, c, :] for c in range(DCH)]

        def load_wqk(p, eng):
            load_wq(p, eng)
            load_wk(p, eng)

        def load_wv(q, eng):
            wvq = w_p.tile([128, DCH, 260], F16, name=f"wv{q}", bufs=1)
            eng.dma_start(out=wvq, in_=wv_d[q])
            wv_ts[q] = [wvq[:, c, :] for c in range(DCH)]

        # ---- small constants first on gpsimd queue (it is free early;
        # all later gpsimd work is broadcasts only) ----
        bqk_t = const.tile([128, 2 * NPAIR], F32)
        nc.gpsimd.dma_start(out=bqk_t, in_=bqk_d[:, :])
        em_t = const.tile([128, SBLK], F32)
        nc.gpsimd.dma_start(out=em_t, in_=em_d[:, :])
        onesr_t = const.tile([1, 128], F16)
        nc.gpsimd.dma_start(out=onesr_t, in_=onesr_d[:, :])
        bvr_t = const.tile([1, NQUAD * 260], F16)
        nc.gpsimd.dma_start(out=bvr_t, in_=bvr_d[:, :])
        bor_t = const.tile([1, D], F16)
        nc.gpsimd.dma_start(out=bor_t, in_=bor_d[:, :])
        gamma1 = const.tile([1, D], F32)
        nc.gpsimd.dma_start(out=gamma1, in_=gamma_d[:, :])
        beta1 = const.tile([1, D], F32)
        nc.gpsimd.dma_start(out=beta1, in_=beta_d[:, :])
        eps_t = const.tile([128, 1], F32)
        nc.vector.memset(eps_t, LN_EPS)
        warm_t = const.tile([128, 512], F16)
        nc.vector.memset(warm_t, 0.0)

        # critical-path first: wq0/wk0 + half-0 x pieces round-robin, then
        # half-1 pieces, then the rest by need
        load_wq(0, nc.sync)
        load_wk(0, nc.scalar)
        load_xt([0, 3], nc.sync, 0)
        load_xt([1, 4], nc.scalar, 0)
        load_xt([2, 5], nc.gpsimd, 0)
        load_xt([0, 3], nc.sync, 1)
        load_xt([1, 4], nc.scalar, 1)
        load_xt([2, 5], nc.gpsimd, 1)
        load_wv(0, nc.scalar)
        load_wqk(1, nc.gpsimd)
        load_wv(1, nc.scalar)
        load_wqk(2, nc.sync)
        load_wqk(3, nc.gpsimd)
        load_wv(2, nc.scalar)
        load_wqk(4, nc.sync)
        load_wqk(5, nc.scalar)
        woa = w_p.tile([128, DCH, D], F16, name="woa", bufs=1)
        nc.sync.dma_start(out=woa[:, 0:3, :], in_=wo_d[:, :3 * D])
        nc.scalar.dma_start(out=woa[:, 3:6, :], in_=wo_d[:, 3 * D:])
        wo_t = [woa[:, c, :] for c in range(DCH)]

        gamma_t = const.tile([128, D], F32)
        nc.gpsimd.partition_broadcast(gamma_t, gamma1)
        beta_t = const.tile([128, D], F32)
        nc.gpsimd.partition_broadcast(beta_t, beta1)

        # ---------- emission helpers ----------
        v_sb = {}   # (quad, sblk) -> [128, 260] f16  (em-scaled, bias incl.)

        def emit_v_block(q, s):
            """V projection for one (quad, s-block): 6 K=128 MMs + rank-one
            bias + one DVE pass (psv * em -> f16)."""
            psv = ps.tile([128, 260], F32, name="psv", tag="proj", bufs=2,
                          padded_shape=[128, 512])
            wv_t = wv_ts[q]
            for c in range(DCH):
                nc.tensor.matmul(psv, xt[c][:, s * 128:(s + 1) * 128], wv_t[c],
                                 start=(c == 0), stop=False)
            nc.tensor.matmul(psv, onesr_t,
                             bvr_t[:, q * 260:(q + 1) * 260],
                             start=False, stop=True)
            vt = v_p.tile([128, 260], F16, name="v_sb", bufs=3 * SBLK)
            nc.vector.tensor_scalar_mul(out=vt, in0=psv,
                                        scalar1=em_t[:, s:s + 1])
            v_sb[(q, s)] = vt

        qt_sb, kt_sb = [None] * NPAIR, [None] * NPAIR

        def emit_proj_half(p, which, half):
            """One half (512 cols) of psq/psk: 6 MMs into an own 1-bank
            psum tile + DVE bias-add into the qt/kt f16 half."""
            w_t = wq_ts[p] if which == "q" else wk_ts[p]
            pst = ps.tile([128, 512], F32,
                          name=("psq" if which == "q" else "psk"),
                          tag="proj", bufs=2)
            for c in range(DCH):
                nc.tensor.matmul(
                    pst, w_t[c], xt[c][:, half * 512:(half + 1) * 512],
                    start=(c == 0), stop=(c == DCH - 1))
            if half == 0:
                t = qk_p.tile([128, S], F16,
                              name=("qt_sb" if which == "q" else "kt_sb"),
                              bufs=4)
                if which == "q":
                    qt_sb[p] = t
                else:
                    kt_sb[p] = t
            t = qt_sb[p] if which == "q" else kt_sb[p]
            col = p if which == "q" else NPAIR + p
            nc.vector.tensor_scalar_add(
                out=t[:, half * 512:(half + 1) * 512], in0=pst,
                scalar1=bqk_t[:, col:col + 1])

        def lookahead_units(p):
            """PE work to interleave into pair p's slot stream, one unit per
            slot.  V tiles for quad q land during pairs 2q-1 / 2q-2; pair 0
            finishes quad 0 (s>=1) with a one-slot lead on its own ctx."""
            units = []
            if p == 0:
                units.append(lambda: emit_proj_half(0, "q", 1))
                units.append(lambda: emit_proj_half(0, "k", 1))
                for s in range(4, SBLK):
                    units.append(lambda s=s: emit_v_block(0, s))
            if 0 < p < 5:
                q = (p + 1) // 2
                s0 = 4 * ((p + 1) % 2)
                for s in range(s0, s0 + 4):
                    units.append(lambda q=q, s=s: emit_v_block(q, s))
            if p + 1 < NPAIR:
                for which in ("q", "k"):
                    for half in range(2):
                        units.append(
                            lambda w=which, h=half: emit_proj_half(
                                p + 1, w, h))
            return units

        def emit_proj_h0_pair0():
            # q/k half-0 for pair 0 with c-chunks interleaved, so both
            # projections stream concurrently as x pieces arrive
            psts = {}
            for which in ("q", "k"):
                psts[which] = ps.tile([128, 512], F32, name="ps" + which,
                                      tag="proj", bufs=2)
            for c in range(DCH):
                for which in ("q", "k"):
                    w_t = wq_ts[0] if which == "q" else wk_ts[0]
                    nc.tensor.matmul(
                        psts[which], w_t[c], xt[c][:, 0:512],
                        start=(c == 0), stop=(c == DCH - 1))
            for which in ("q", "k"):
                t = qk_p.tile([128, S], F16,
                              name=("qt_sb" if which == "q" else "kt_sb"),
                              bufs=4)
                if which == "q":
                    qt_sb[0] = t
                else:
                    kt_sb[0] = t
                col = 0 if which == "q" else NPAIR
                nc.vector.tensor_scalar_add(
                    out=t[:, 0:512], in0=psts[which],
                    scalar1=bqk_t[:, col:col + 1])

        # ---- HAM warm-up: dummy matmuls on memset data keep the PE clock
        # at 8/8 through the initial DMA wait, so the first real
        # projections run at full speed (results are never read) ----
        warm_ps = ps.tile([128, 512], F32, name="warm_ps", tag="proj",
                          bufs=2)
        for _ in range(30):
            nc.tensor.matmul(warm_ps, warm_t[:, 0:128], warm_t,
                             start=True, stop=True)

        # ---- prologue: pair 0 projections (half-0 first) + the v tiles
        # that only need xt half-0 (they fill the DMA wait); q/k half-1
        # moves into pair 0's unit stream ----
        emit_proj_h0_pair0()
        for s in range(4):
            emit_v_block(0, s)

        # ---- pair loop ----
        ctxt = []           # per pair [128, 1024] f16 ctx^T (unnormalized
                            # at first; normalized in place by gpsimd)
        pending = []        # deferred (fn) emissions, flushed at slot starts

        for p in range(NPAIR):
            qt, kt = qt_sb[p], kt_sb[p]
            quad, l0 = divmod(2 * p, 4)

            ct = cx_p.tile([128, S], F16, name="ctxt", bufs=NPAIR)
            ctxt.append(ct)

            units = lookahead_units(p)
            ui = 0
            for iblk in range(2):
                pcx = [ps.tile([65, 512], F32, name="pscx", tag="cx", bufs=2)
                       for _ in range(2)]
                ets = [None] * SBLK

                def emit_ctx(j, pcx=pcx, ets=ets, quad=quad, l0=l0):
                    for idx in range(2):
                        vsl = v_sb[(quad, j)][:, (l0 + idx) * 65:
                                              (l0 + idx + 1) * 65]
                        nc.tensor.matmul(pcx[idx], vsl,
                                         ets[j][:, idx * 512:(idx + 1) * 512],
                                         start=(j == 0), stop=(j == SBLK - 1))

                for j in range(SBLK):
                    # flush deferred (previous iblk's normalize) work; at
                    # j==2 the broadcast DMA has had time to land
                    if j == 2:
                        while pending:
                            pending.pop(0)()
                    # scores for (iblk, j): two heads row-tiled
                    pst = ps.tile([128, 1024], F32, name="psst", tag="st",
                                  bufs=2)
                    nc.tensor.matmul(
                        pst[:, 0:512], kt[0:64, j * 128:(j + 1) * 128],
                        qt[0:64, iblk * 512:(iblk + 1) * 512],
                        start=True, stop=True, tile_position=(0, 0))
                    nc.tensor.matmul(
                        pst[:, 512:1024], kt[64:128, j * 128:(j + 1) * 128],
                        qt[64:128, iblk * 512:(iblk + 1) * 512],
                        start=True, stop=True, tile_position=(64, 0))
                    # exp straight from PSUM, no bias (mask lives in V)
                    et = e_p.tile([128, 1024], F16, name="expt", bufs=6)
                    nc.scalar.activation(et, pst, AF.Exp)
                    ets[j] = et
                    # lookahead PE work rides the exp latency (slot 7's unit
                    # is deferred past the epilogue)
                    if ui < len(units) and j < SBLK - 1:
                        units[ui]()
                        ui += 1
                    # ctx runs two slots behind so boundary DVE copies never
                    # block the scores/exp stream
                    if j > 1:
                        emit_ctx(j - 2)
                emit_ctx(SBLK - 2)
                emit_ctx(SBLK - 1)
                # iblk epilogue: denominators + ctx copy-out (release PSUM
                # fast); normalization is deferred off the critical path.
                # pcx release chain first (drow rows + ct casts), with a
                # priority boost over same-window unit DVE work; the recip
                # only feeds the deferred normalize, so it runs after
                drow = z_p.tile([1, 1024], F32, name="drow", bufs=4)
                with tc.high_priority(offset=40):
                    for idx in range(2):
                        nc.vector.tensor_copy(
                            out=drow[0:1, idx * 512:(idx + 1) * 512],
                            in_=pcx[idx][64:65, :])
                    for idx in range(2):
                        nc.vector.tensor_copy(
                            out=ct[idx * 64:(idx + 1) * 64,
                                   iblk * 512:(iblk + 1) * 512],
                            in_=pcx[idx][0:64, :])
                rinv = z_p.tile([1, 1024], F32, name="rinv", bufs=4)
                nc.vector.reciprocal_approx_fast(out=rinv, in_=drow)
                if p == NPAIR - 1:
                    # pair 5 has no lookahead units: a few dummy matmuls
                    # cover the boundary DVE copies so the PE (and HAM)
                    # never go idle while the release chain runs
                    dps = ps.tile([128, 512], F32, name="dummy_ps",
                                  tag="proj", bufs=2)
                    for _ in range(4):
                        nc.tensor.matmul(dps, warm_t[:, 0:128], warm_t,
                                         start=True, stop=True)

                def norm_chain(ct=ct, rinv=rinv, iblk=iblk):
                    bc = z_p.tile([128, 1024], F32, name="bc", bufs=3)
                    nc.gpsimd.partition_broadcast(bc, rinv)
                    for idx in range(2):
                        csl = ct[idx * 64:(idx + 1) * 64,
                                 iblk * 512:(iblk + 1) * 512]
                        nc.vector.tensor_mul(
                            out=csl, in0=csl,
                            in1=bc[idx * 64:(idx + 1) * 64,
                                   idx * 512:(idx + 1) * 512])
                pending.append(norm_chain)

            # drain remaining lookahead units at pair end
            while ui < len(units):
                units[ui]()
                ui += 1

        while pending:
            pending.pop(0)()

        # ---- output projection + layernorm, per row block ----
        for s in range(SBLK):
            z0 = z_p.tile([128, D], F32, name="z0_sb", bufs=3)
            if s == 0:
                # s=0 uses the freed cx banks so it need not wait for the
                # last exps to release an st buffer
                halves = (
                    (0, 512, ps.tile([128, 512], F32, name="pso0a",
                                     tag="cx", bufs=2)),
                    (512, 768, ps.tile([128, 256], F32, name="pso0b",
                                       tag="cx", bufs=2,
                                       padded_shape=[128, 512])))
            else:
                # st tag double-buffers pso; the LN chain (~3us) fits
                # within two outproj slots so depth 2 does not stall
                pso = ps.tile([128, D], F32, name="pso", tag="st", bufs=2,
                              padded_shape=[128, 1024])
                halves = ((0, 512, pso[:, 0:512]),
                          (512, 768, pso[:, 512:768]))
            for d0, d1, tgt in halves:
                for p in range(NPAIR):
                    nc.tensor.matmul(
                        tgt,
                        ctxt[p][:, s * 128:(s + 1) * 128],
                        wo_t[p][:, d0:d1],
                        start=(p == 0), stop=False)
                # + bo via a K=1 rank-one update: ones_col x bo_row
                nc.tensor.matmul(tgt, onesr_t, bor_t[:, d0:d1],
                                 start=False, stop=True)
            # evict psum fast (one DVE copy per tile) so the rotation
            # never stalls; the LN chain runs off the SBUF copy
            if s == 0:
                for d0, d1, tgt in halves:
                    nc.vector.tensor_copy(out=z0[:, d0:d1], in_=tgt)
            else:
                nc.vector.tensor_copy(out=z0, in_=pso)
            stats = z_p.tile([128, 3, 6], F32, name="stats", bufs=3)
            for g in range(3):
                nc.vector.bn_stats(out=stats[:, g, :],
                                   in_=z0[:, g * 256:(g + 1) * 256])
            mv = z_p.tile([128, 2], F32, name="mv", bufs=3)
            nc.vector.bn_aggr(out=mv, in_=stats)
            stdv = z_p.tile([128, 1], F32, name="stdv", bufs=3)
            nc.scalar.activation(stdv, mv[:, 1:2], AF.Sqrt, bias=eps_t)
            rstd = z_p.tile([128, 1], F32, name="rstd", bufs=3)
            nc.vector.reciprocal(out=rstd, in_=stdv)
            nmr = z_p.tile([128, 1], F32, name="nmr", bufs=3)
            nc.vector.tensor_scalar(out=nmr, in0=mv[:, 0:1], scalar1=rstd,
                                    scalar2=-1.0, op0=mybir.AluOpType.mult,
                                    op1=mybir.AluOpType.mult)
            z = z_p.tile([128, D], F32, name="z_sb", bufs=3)
            nc.scalar.activation(z, z0, AF.Identity, bias=nmr, scale=rstd)
            nc.vector.tensor_mul(out=z, in0=z, in1=gamma_t)
            nc.gpsimd.tensor_add(out=z, in0=z, in1=beta_t)
            eng = (nc.sync, nc.scalar)[s % 2]
            eng.dma_start(out=out_d[s * 128:(s + 1) * 128, :], in_=z)

    nc.compile()
    return nc


def _host_inputs(inputs):
    x = np.asarray(inputs["input_tensor"], np.float32)
    mask = np.asarray(inputs["attention_mask"])
    Wq = np.asarray(inputs["Wq"], np.float32)
    bq = np.asarray(inputs["bq"], np.float32)
    Wk = np.asarray(inputs["Wk"], np.float32)
    bk = np.asarray(inputs["bk"], np.float32)
    Wv = np.asarray(inputs["Wv"], np.float32)
    bv = np.asarray(inputs["bv"], np.float32)
    Wo = np.asarray(inputs["Wo"], np.float32)
    bo = np.asarray(inputs["bo"], np.float32)
    gamma = np.asarray(inputs["gamma"], np.float32)
    beta = np.asarray(inputs["beta"], np.float32)

    scale = np.float32(1.0 / np.sqrt(DH))
    wq_flat = np.ascontiguousarray(
        (Wq * scale).transpose(1, 0, 2).reshape(D, D))
    wk_flat = np.ascontiguousarray(Wk.transpose(1, 0, 2).reshape(D, D))
    bq_s = (bq * scale).reshape(D)
    bk_s = bk.reshape(D)

    wv_aug = np.zeros((D, NQUAD * 260), np.float32)
    bv_aug = np.zeros((1, NQUAD * 260), np.float32)
    for h in range(H):
        q, l = divmod(h, 4)
        base = q * 260 + l * 65
        wv_aug[:, base:base + 64] = Wv[h]
        bv_aug[0, base:base + 64] = bv[h]
        bv_aug[0, base + 64] = 1.0

    bqk = np.zeros((128, 2 * NPAIR), np.float32)
    for p in range(NPAIR):
        bqk[:, p] = bq_s[p * 128:(p + 1) * 128]
        bqk[:, NPAIR + p] = bk_s[p * 128:(p + 1) * 128]

    def sbuf_layout(w, width):
        # [D, n*width] -> [n, 128, DCH*width]: partition-major per tile
        n = w.shape[1] // width
        return np.ascontiguousarray(
            w.reshape(DCH, 128, n, width).transpose(2, 1, 0, 3).reshape(
                n, 128, DCH * width).astype(np.float16))

    shared = {
        "wq": sbuf_layout(wq_flat, 128), "wk": sbuf_layout(wk_flat, 128),
        "wv": sbuf_layout(wv_aug, 260),
        "wo": sbuf_layout(np.ascontiguousarray(Wo), D)[0],
        "bqk": bqk, "bvr": bv_aug.astype(np.float16),
        "gamma": gamma.reshape(1, D), "beta": beta.reshape(1, D),
        "onesr": np.ones((1, 128), np.float16),
        "bor": bo.reshape(1, D).astype(np.float16),
    }
    in_maps = []
    for b in range(B):
        em = mask[b].astype(np.float32)  # 1.0 keep, 0.0 masked
        in_maps.append({
            **shared,
            "xt": np.ascontiguousarray(
                x[b].T.reshape(DCH, 128, S).transpose(1, 0, 2).reshape(
                    128, DCH * S).astype(np.float16)),
            "em": np.ascontiguousarray(em.reshape(SBLK, 128).T),
        })
    return in_maps


def _get_program():
    global _PROGRAM
    if _PROGRAM is None:
        _PROGRAM = _build_program()
    return _PROGRAM


def kernel(**inputs):
    from concourse.bass_utils import run_bass_kernel_spmd

    nc = _get_program()
    in_maps = _host_inputs(inputs)
    res = run_bass_kernel_spmd(nc, in_maps, list(range(B)))
    return np.stack([res.results[b]["out"] for b in range(B)], axis=0)


if __name__ == "__main__":
    rng = np.random.default_rng(0)
    demo = {
        "input_tensor": rng.standard_normal((B, S, D)).astype(np.float32),
        "attention_mask": np.ones((B, S), bool),
        "Wq": rng.standard_normal((H, D, DH)).astype(np.float32) * 0.03,
        "bq": rng.standard_normal((H, DH)).astype(np.float32) * 0.03,
        "Wk": rng.standard_normal((H, D, DH)).astype(np.float32) * 0.03,
        "bk": rng.standard_normal((H, DH)).astype(np.float32) * 0.03,
        "Wv": rng.standard_normal((H, D, DH)).astype(np.float32) * 0.03,
        "bv": rng.standard_normal((H, DH)).astype(np.float32) * 0.03,
        "Wo": rng.standard_normal((D, D)).astype(np.float32) * 0.03,
        "bo": rng.standard_normal((D,)).astype(np.float32) * 0.03,
        "gamma": np.ones((D,), np.float32),
        "beta": np.zeros((D,), np.float32),
    }
    out = kernel(**demo)
    print("kernel ran, out shape", out.shape, "finite:", np.isfinite(out).all())
